# revision 46
# baseline (speedup 1.0000x reference)
"""Distributed GQA attention kernel for 8 Trainium2 NeuronCores.

Problem: B=1, S=2048, DIM=2048, 32 Q heads / 8 KV heads, head_dim 64,
partial rotate-half RoPE over first 32 dims, causal softmax, out
projection.

Sharding (tensor-parallel over heads, fully collective-free):
- Core h owns one GQA group: Q heads [4h, 4h+4) and KV head h, i.e. a
  wqkv column shard (2048, 384) and a wo ROW shard (256, 2048).
- All activations live in transposed (feature, seq) layout; the host
  feeds x^T once, so qkv^T = w^T-tiles @ x^T comes out feature-major.
- QKV runs kt-OUTER so each xT k-tile is fully consumed as its DMA
  lands: pass A = (kv | q01) in 8 psum accumulators, pass B = q23 after
  the RoPE of k/q01 kicks off on DVE, so the PE never waits on HBM.
- v^T is built with one XBAR dma_start_transpose (no PE transposes);
  the 65th lhsT column of ones makes PV emit the softmax denominator.
- Scores for a head pair run CONCURRENTLY in disjoint PE row groups
  (K=64 each): head-o uses an explicit tile_position=(64, 0) so no
  duplicated k buffer is needed.
- Chunks processed DESCENDING (c=3..0) in a flat software pipeline with
  a 2-t-tile A-lead; per-head normalization = DVE reciprocal of the
  denominator row + a K=1 f32r PE matmul that broadcasts it across 64
  partitions into a spare psum slot + one DVE multiply (no gpsimd, no
  DRAM round trips).
- Out projection is ROW-parallel: partials in bf16, summed on host.
  Groups are interleaved into the pipeline (2 per slot under backlog)
  so the tail is short.
"""

import os
import sys
import types
import numpy as np
import ml_dtypes

BF16 = ml_dtypes.bfloat16

S = 2048
DIM = 2048
N_HEAD = 32
N_KV = 8
HEAD_DIM = 64
ROPE = 32
N_CORES = 8
QH_PER_CORE = N_HEAD // N_KV          # 4 query heads per core
QCOLS = QH_PER_CORE * HEAD_DIM        # 256 q columns per core
WSH = QCOLS + 2 * HEAD_DIM            # 384 wqkv shard columns
CW = 512                              # s_q chunk width for attention
NCH = S // CW                         # chunks
KT = DIM // 128                       # 16 k tiles for dense matmuls

_COMPILED = None


def _install_ntff_hook():
    """Shim antenv.axon_hooks so bass_utils can NTFF-profile under axon."""
    try:
        import antenv
        if 'antenv.axon_hooks' in sys.modules:
            return
        mod = types.ModuleType('antenv.axon_hooks')
        mod._hook = None

        def set_axon_ntff_profile_hook(h):
            mod._hook = h

        def get_axon_ntff_profile_hook():
            return mod._hook

        mod.set_axon_ntff_profile_hook = set_axon_ntff_profile_hook
        mod.get_axon_ntff_profile_hook = get_axon_ntff_profile_hook
        sys.modules['antenv.axon_hooks'] = mod
        antenv.axon_hooks = mod
        try:
            from trn_agent_boot.trn_boot import _ntff_profile_via_ctypes
            hook = _ntff_profile_via_ctypes('/opt/axon/libaxon_pjrt.so')
            if hook is not None:
                mod._hook = hook
        except Exception:
            pass
    except Exception:
        pass


def build_kernel():
    import concourse.bass as bass
    import concourse.mybir as mybir
    import concourse.tile as tile
    from concourse import bacc

    bf = mybir.dt.bfloat16
    f32 = mybir.dt.float32
    f32r = mybir.dt.float32r
    MUL = mybir.AluOpType.mult
    ADD = mybir.AluOpType.add
    EXP = mybir.ActivationFunctionType.Exp

    nc = bacc.Bacc('TRN2', target_bir_lowering=False, debug=False,
                   num_devices=N_CORES)

    xT = nc.dram_tensor('xT', [DIM, S], bf, kind='ExternalInput')
    wqkv = nc.dram_tensor('wqkv', [DIM, WSH], bf, kind='ExternalInput')
    wo = nc.dram_tensor('wo', [QCOLS, DIM], bf, kind='ExternalInput')
    cosf = nc.dram_tensor('cosf', [128, S], bf, kind='ExternalInput')
    sinf = nc.dram_tensor('sinf', [128, S], bf, kind='ExternalInput')
    maskd = nc.dram_tensor('maskd', [128, 128], bf, kind='ExternalInput')
    onesd = nc.dram_tensor('onesd', [1, 64], f32r, kind='ExternalInput')
    out_ext = nc.dram_tensor('out', [DIM, S], bf, kind='ExternalOutput')

    with tile.TileContext(nc) as tc:
        with (
            tc.tile_pool(name='const', bufs=1) as const_pool,
            tc.tile_pool(name='persist', bufs=1) as persist,
        ):
            # ---- constants ----
            cos_sb = const_pool.tile([128, S], bf)
            sin_sb = const_pool.tile([128, S], bf)
            mask_sb = const_pool.tile([128, 128], bf)
            # ones row at partition 64 so it aligns with the pv
            # denominator row for the K=1 broadcast matmul
            ones_r = const_pool.tile([65, 64], f32r)
            w_sb = persist.tile([128, KT, WSH], bf)
            wo_sb = persist.tile([128, QCOLS // 128, DIM], bf)

            # m tile 0 -> q heads 0,1 ; 1 -> q heads 2,3 ; 2 -> [k | v]
            qkvT = [persist.tile([128, S], bf, name=f'qkvT{m}')
                    for m in range(3)]
            v_aug = persist.tile([128, S // 128, HEAD_DIM + 1], bf)
            vT_scr = persist.tile([128, S // 128, HEAD_DIM], bf)
            kk = persist.tile([128, S], bf)
            # shared RoPE rotation scratch: zeroed ONCE while DVE is
            # idle; non-rope rows stay zero through all three uses
            # (each sin-multiply rewrites them as 0 * 0)
            rot = persist.tile([128, S], bf)
            nc.vector.memset(rot[:], 0.0)

            # ---- qkvT = (x @ w_shard)^T, kt-outer two-pass ----
            with (
                tc.tile_pool(name='xt_pool', bufs=1) as xt_pool,
                tc.tile_pool(name='qkv_psum', bufs=1, space='PSUM') as qp,
            ):
                xt_sb = xt_pool.tile([128, KT, S], bf)
                # first two xt tiles on sync AHEAD of the weight loads
                # so the very first matmuls start ~6us earlier; the rest
                # stream on the gpsimd queue in parallel
                for kt in (0, 1):
                    nc.sync.dma_start(
                        xt_sb[:, kt, :], xT[kt * 128:(kt + 1) * 128, :])
                for kt in range(2, KT):
                    nc.gpsimd.dma_start(
                        xt_sb[:, kt, :], xT[kt * 128:(kt + 1) * 128, :])
                for kt in range(KT):
                    nc.sync.dma_start(w_sb[:, kt, :],
                                      wqkv[kt * 128:(kt + 1) * 128, :])
                nc.sync.dma_start(cos_sb[:], cosf[:])
                nc.sync.dma_start(sin_sb[:], sinf[:])
                nc.sync.dma_start(mask_sb[:], maskd[:])
                nc.sync.dma_start(
                    wo_sb[:], wo[:].rearrange('(o p) n -> p o n', p=128))
                nc.sync.dma_start(ones_r[HEAD_DIM:HEAD_DIM + 1, :],
                                  onesd[:])

                # pass A: kv (m=2) + q01 (m=0), 8 accumulators, consume
                # each xt tile fully as it arrives from HBM
                pass_a = [(2, sc) for sc in range(4)] + \
                         [(0, sc) for sc in range(4)]
                psA = {}
                for idx, (m, sc) in enumerate(pass_a):
                    psA[(m, sc)] = qp.tile([128, 512], f32,
                                           tag=f'acc{idx}', name=f'ps{idx}')
                for kt in range(KT):
                    for m, sc in pass_a:
                        nc.tensor.matmul(
                            psA[(m, sc)][:],
                            lhsT=w_sb[:, kt, m * 128:(m + 1) * 128],
                            rhs=xt_sb[:, kt, sc * 512:(sc + 1) * 512],
                            start=(kt == 0), stop=(kt == KT - 1))

                def rope_q_sc(qt, sc):
                    cs = slice(sc * 512, (sc + 1) * 512)
                    for b in (0, 64):
                        nc.sync.dma_start(rot[b:b + 16, cs],
                                          qt[b + 16:b + 32, cs])
                        nc.sync.dma_start(rot[b + 16:b + 32, cs],
                                          qt[b:b + 16, cs])
                    nc.vector.tensor_tensor(rot[:, cs], rot[:, cs],
                                            sin_sb[:, cs], MUL)
                    nc.vector.tensor_tensor(qt[:, cs], qt[:, cs],
                                            cos_sb[:, cs], MUL)
                    nc.vector.tensor_tensor(qt[:, cs], qt[:, cs],
                                            rot[:, cs], ADD)

                # kv psums out first, per 512-col slice ASCENDING (the
                # first scores lhsT needs k tile 0): copy, RoPE-k, dup
                kvt = qkvT[2]
                for sc in range(4):
                    cs = slice(sc * 512, (sc + 1) * 512)
                    nc.vector.tensor_copy(kvt[:, cs], psA[(2, sc)][:])
                    nc.sync.dma_start(rot[0:16, cs], kvt[16:32, cs])
                    nc.sync.dma_start(rot[16:32, cs], kvt[0:16, cs])
                    nc.vector.tensor_tensor(rot[0:64, cs], rot[0:64, cs],
                                            sin_sb[0:64, cs], MUL)
                    nc.vector.tensor_tensor(kvt[0:64, cs], kvt[0:64, cs],
                                            cos_sb[0:64, cs], MUL)
                    nc.vector.tensor_tensor(kvt[0:64, cs], kvt[0:64, cs],
                                            rot[0:64, cs], ADD)
                    # k duplicated to both partition halves so paired
                    # scores lhsT matches each q head's base partition
                    nc.sync.dma_start(kk[0:64, cs], kvt[0:64, cs])
                    nc.sync.dma_start(kk[64:128, cs], kvt[0:64, cs])
                # v^T via XBAR dma transpose into contiguous scratch,
                # then gpsimd repack next to the ones column
                nc.gpsimd.memset(v_aug[:], 1.0)
                nc.sync.dma_start_transpose(vT_scr[:], kvt[64:128, :])
                nc.gpsimd.tensor_copy(v_aug[:, :, 0:HEAD_DIM], vT_scr[:])

                # q01 slices DESCENDING: chunk 3 is processed first and
                # stage (c, hp) reads only q columns of slice c
                for sc in (3, 2, 1, 0):
                    nc.vector.tensor_copy(
                        qkvT[0][:, sc * 512:(sc + 1) * 512], psA[(0, sc)][:])
                    rope_q_sc(qkvT[0], sc)

                # pass B: q23 (m=1); reuses acc0..3 (freed by kv copies).
                # Runs on PE while DVE does RoPE-k/q01.
                psB = [qp.tile([128, 512], f32, tag=f'acc{sc}',
                               name=f'psB{sc}') for sc in range(4)]
                for kt in range(KT):
                    for sc in range(4):
                        nc.tensor.matmul(
                            psB[sc][:],
                            lhsT=w_sb[:, kt, 128:256],
                            rhs=xt_sb[:, kt, sc * 512:(sc + 1) * 512],
                            start=(kt == 0), stop=(kt == KT - 1))
                for sc in (3, 2, 1, 0):
                    nc.vector.tensor_copy(
                        qkvT[1][:, sc * 512:(sc + 1) * 512], psB[sc][:])
                    rope_q_sc(qkvT[1], sc)

            # ---- attention + out projection, descending chunks ----
            import contextlib
            sbuf_pools = contextlib.ExitStack()
            probs_pool = sbuf_pools.enter_context(
                tc.tile_pool(name='probs', bufs=22))
            smax_pool = sbuf_pools.enter_context(
                tc.tile_pool(name='smax', bufs=3))
            attn_pool = sbuf_pools.enter_context(
                tc.tile_pool(name='attn_sb', bufs=4))
            attnL_pool = sbuf_pools.enter_context(
                tc.tile_pool(name='attnL', bufs=2))
            outp = sbuf_pools.enter_context(
                tc.tile_pool(name='out_sb', bufs=4))
            with (
                tc.tile_pool(name='sc_psum', bufs=2, space='PSUM') as scp,
                tc.tile_pool(name='pv_psum', bufs=1, space='PSUM') as pvp,
                tc.tile_pool(name='wo_psum', bufs=2, space='PSUM') as wop,
            ):
                def a_pair_step(c, hp, tt, probs_list):
                    """paired scores + single exp for (chunk c, pair hp)."""
                    q_tile = qkvT[hp]
                    start = max(0, 128 * tt - CW * c)
                    ps = scp.tile([128, 2 * CW], f32, tag='scps', name='ps')
                    for qp0, off in ((0, 0), (64, CW)):
                        nc.tensor.matmul(
                            ps[:, off + start:off + CW],
                            lhsT=kk[qp0:qp0 + 64, tt * 128:(tt + 1) * 128],
                            rhs=q_tile[qp0:qp0 + 64,
                                       c * CW + start:(c + 1) * CW],
                            start=True, stop=True)
                    probs = probs_pool.tile([128, 2 * CW], bf, tag='pb',
                                            name='probs')
                    nc.scalar.activation(
                        probs[:, start:2 * CW], ps[:, start:2 * CW],
                        EXP, scale=0.125)
                    if 128 * tt >= CW * c:  # diagonal tile
                        for off in (0, CW):
                            nc.vector.tensor_tensor(
                                probs[:, off + start:off + start + 128],
                                probs[:, off + start:off + start + 128],
                                mask_sb[:], MUL)
                    probs_list.append(probs)

                def b_pair_step(c, tt, n_tt, pvs, probs):
                    # rhs trimmed to the causally-valid columns; tt=0
                    # (start=0) initializes the full accumulator width
                    start = max(0, 128 * tt - CW * c)
                    for i, pv in enumerate(pvs):
                        nc.tensor.matmul(
                            pv[:, start:CW],
                            lhsT=v_aug[:, tt, :],
                            rhs=probs[:, i * CW + start:(i + 1) * CW],
                            start=(tt == 0), stop=(tt == n_tt - 1),
                            skip_group_check=True)

                def norm_pair(c, hp, pvs, attnL):
                    """divide both heads by their denominator rows: one
                    shared partition-reshaped DVE recip + per-head K=1
                    f32r PE broadcast into a wop psum + DVE mult. The
                    [1,2CW]->[128,2CW/128] reshape keeps the iterative
                    reciprocal off the single-lane path (a [1,512]
                    recip costs ~4us on DVE)."""
                    den = smax_pool.tile([HEAD_DIM + 1, 2, CW], f32,
                                         tag='den', name='den')
                    for k in range(2):
                        nc.vector.tensor_copy(
                            den[HEAD_DIM:HEAD_DIM + 1, k, :],
                            pvs[k][HEAD_DIM:HEAD_DIM + 1, :])
                    den_p = smax_pool.tile([128, 2 * CW // 128], f32,
                                           tag='denp', name='den_p')
                    nc.gpsimd.dma_start(
                        den_p[:], den[HEAD_DIM:HEAD_DIM + 1, :, :])
                    rec_p = smax_pool.tile([128, 2 * CW // 128], f32,
                                           tag='recp', name='rec_p')
                    nc.vector.reciprocal(rec_p[:], den_p[:])
                    rec_t = smax_pool.tile([65, 2, CW], f32r,
                                           tag='recrow', name='rec_t')
                    nc.gpsimd.dma_start(
                        rec_t[HEAD_DIM:HEAD_DIM + 1, :, :],
                        rec_p[:].bitcast(f32r))
                    for k in range(2):
                        pv = pvs[k]
                        attn_un = attn_pool.tile([64, CW], bf, tag='attnu',
                                                 name='attn_un')
                        nc.vector.tensor_copy(attn_un[:],
                                              pv[0:HEAD_DIM, :])
                        bc = wop.tile([128, CW], f32, tag='wops',
                                      name='bc')
                        nc.tensor.matmul(
                            bc[0:64, :],
                            lhsT=ones_r[HEAD_DIM:HEAD_DIM + 1, :],
                            rhs=rec_t[HEAD_DIM:HEAD_DIM + 1, k, :],
                            start=True, stop=True, skip_group_check=True)
                        if k == 0:
                            nc.vector.tensor_tensor(
                                attnL[0:64, hp, :], attn_un[:],
                                bc[0:64, :], MUL)
                        else:
                            # odd head: normalize then partition-shift
                            attn_n = attn_pool.tile([64, CW], bf,
                                                    tag='attnn',
                                                    name='attn_n')
                            nc.vector.tensor_tensor(
                                attn_n[:], attn_un[:], bc[0:64, :], MUL)
                            nc.sync.dma_start(attnL[64:128, hp, :],
                                              attn_n[:])

                # row-parallel partial out projection in bf16; groups are
                # interleaved into the pipeline to fill PE bubbles.
                pending_out = []
                emitted_n = [0]

                def emit_out_group(pool=None, drain=False):
                    if not pending_out:
                        return
                    c, et = pending_out.pop(0)
                    emitted_n[0] += 1
                    attnL = attnL_of[c]
                    pso = (pool or wop).tile([128, CW], f32, tag='wops',
                                             name='pso')
                    for ft in range(QCOLS // 128):
                        nc.tensor.matmul(
                            pso[:],
                            lhsT=wo_sb[:, ft, et * 128:(et + 1) * 128],
                            rhs=attnL[:, ft, :],
                            start=(ft == 0),
                            stop=(ft == QCOLS // 128 - 1))
                    osb = outp.tile([128, CW], bf, tag='osb', name='osb')
                    # in the drain (exp done) the scalar engine is idle:
                    # alternate 1:1 so the wop ring turns faster
                    on_act = (emitted_n[0] % 2 == 0) if drain \
                        else (et % 4 == 3)
                    if not on_act:
                        nc.vector.tensor_copy(osb[:], pso[:])
                    else:
                        nc.scalar.activation(
                            osb[:], pso[:],
                            mybir.ActivationFunctionType.Copy)
                    nc.sync.dma_start(
                        out_ext[et * 128:(et + 1) * 128,
                                c * CW:(c + 1) * CW], osb[:])

                # flat software pipeline, chunks DESCENDING, A leads B by
                # LEAD t-tiles; norms of stage i-2 are emitted 2 slots
                # into stage i so their DVE chain hides under PE work.
                LEAD = 3
                NP = QH_PER_CORE // 2  # head pairs per core
                stages = [(c, hp) for c in range(NCH - 1, -1, -1)
                          for hp in range(NP)]
                n_tt_of = lambda c: (CW // 128) * (c + 1)
                probs_by_stage = {}
                pv_by_stage = {}
                attnL_of = {}
                pending_norm = []
                for c in range(NCH):
                    attnL_of[c] = attnL_pool.tile(
                        [128, NP, CW], bf, tag='attnL', name=f'attnL{c}')

                def emit_norms():
                    while pending_norm:
                        fn = pending_norm.pop(0)
                        fn()

                for i in range(len(stages) + 1):
                    cur = stages[i] if i < len(stages) else None
                    prev = stages[i - 1] if i >= 1 else None
                    if cur is not None:
                        probs_by_stage[cur] = []
                    if prev is not None:
                        pv_by_stage[prev] = [
                            pvp.tile([HEAD_DIM + 1, CW], f32,
                                     tag=f'pv{k}', name=f'pv_{k}')
                            for k in range(2)]
                    na = n_tt_of(cur[0]) if cur is not None else 0
                    nb = n_tt_of(prev[0]) if prev is not None else 0
                    steps = max(na, nb + LEAD) if prev is not None else na
                    for j in range(steps):
                        if cur is not None and j < na:
                            a_pair_step(cur[0], cur[1], j,
                                        probs_by_stage[cur])
                        if j == 0:
                            emit_norms()
                        if prev is not None and 0 <= j - LEAD < nb:
                            b_pair_step(prev[0], j - LEAD, nb,
                                        pv_by_stage[prev],
                                        probs_by_stage[prev][j - LEAD])
                        emit_out_group()
                        if len(pending_out) >= 12 or \
                                (i >= len(stages) - 2 and pending_out):
                            emit_out_group()
                    if prev is None:
                        continue
                    pc, php = prev

                    def queue_norm(pc=pc, php=php, pvs=pv_by_stage[prev]):
                        norm_pair(pc, php, pvs, attnL_of[pc])
                        if php == NP - 1:
                            for et in range(DIM // 128):
                                pending_out.append((pc, et))
                    pending_norm.append(queue_norm)
                emit_norms()

            # psum pools above are dead now: drain the remaining out
            # groups through a deep dedicated ring so semaphore latency
            # is fully hidden
            with tc.tile_pool(name='drain_psum', bufs=8,
                              space='PSUM') as dpool:
                while pending_out:
                    emit_out_group(pool=dpool, drain=True)
            sbuf_pools.close()

    nc.compile()
    return nc


def _prepare_in_maps(x, cos, sin, wqkv, wo):
    x2 = np.ascontiguousarray(np.asarray(x, dtype=np.float32).reshape(S, DIM))
    xT = np.ascontiguousarray(x2.T).astype(BF16)
    cos2 = np.asarray(cos, dtype=np.float32).reshape(S, ROPE)
    sin2 = np.asarray(sin, dtype=np.float32).reshape(S, ROPE)
    cosT = np.ascontiguousarray(cos2.T)  # (32, S)
    sinT = np.ascontiguousarray(sin2.T)

    # cos_full: blocks of 64 rows: [cos(32) | ones(32)] twice
    cos_full = np.ones((128, S), dtype=np.float32)
    sin_full = np.zeros((128, S), dtype=np.float32)
    for b in (0, 64):
        cos_full[b:b + 32] = cosT
        sin_full[b:b + 16] = -sinT[0:16]
        sin_full[b + 16:b + 32] = sinT[16:32]
    cos_full = cos_full.astype(BF16)
    sin_full = sin_full.astype(BF16)

    # lower-triangle-inclusive mask for diagonal 128x128 blocks:
    # keep (p, f) iff f >= p
    mask = (np.arange(128)[None, :] >= np.arange(128)[:, None])
    mask = mask.astype(BF16)

    wq = np.asarray(wqkv, dtype=np.float32)
    wov = np.asarray(wo, dtype=np.float32)
    in_maps = []
    for h in range(N_CORES):
        w_shard = np.concatenate([
            wq[:, h * QCOLS:(h + 1) * QCOLS],
            wq[:, DIM + h * HEAD_DIM:DIM + (h + 1) * HEAD_DIM],
            wq[:, DIM + N_KV * HEAD_DIM + h * HEAD_DIM:
               DIM + N_KV * HEAD_DIM + (h + 1) * HEAD_DIM],
        ], axis=1).astype(BF16)
        wo_shard = np.ascontiguousarray(
            wov[h * QCOLS:(h + 1) * QCOLS, :]).astype(BF16)
        in_maps.append({
            'xT': xT,
            'wqkv': np.ascontiguousarray(w_shard),
            'wo': wo_shard,
            'cosf': cos_full,
            'sinf': sin_full,
            'maskd': np.ascontiguousarray(mask),
            'onesd': np.ones((1, 64), dtype=np.float32),
        })
    return in_maps


def kernel(x, cos, sin, wqkv, wo):
    global _COMPILED
    from concourse.bass_utils import run_bass_kernel_spmd

    _install_ntff_hook()
    if _COMPILED is None:
        _COMPILED = build_kernel()
    nc = _COMPILED

    in_maps = _prepare_in_maps(x, cos, sin, wqkv, wo)
    trace = bool(os.environ.get('BASS_KERNEL_TRACE'))
    tmpdir = os.environ.get('BASS_KERNEL_TRACE_DIR') or None
    res = run_bass_kernel_spmd(nc, in_maps, list(range(N_CORES)),
                               trace=trace, tmpdir=tmpdir)
    if trace:
        kernel.last_exec_time_ns = res.exec_time_ns

    outT = np.zeros((DIM, S), dtype=np.float32)
    for h in range(N_CORES):
        outT += np.asarray(res.results[h]['out'], dtype=np.float32)
    return np.ascontiguousarray(outT.T).reshape(1, S, DIM)


kernel.last_exec_time_ns = None
